# revision 60
# baseline (speedup 1.0000x reference)
"""Trainium2 Bass kernel: causal sliding-window attention block (V2, fp8).

Model (see reference): x:[2,2048,512] -> q/k/v proj (8 heads x 64) ->
causal sliding-window attention (W=128) -> out proj.

Sharding: 8 cores = 2 batches x 4 sequence chunks of 512 rows.
Each core gets a 640-row halo slice of x, all weight matrices, and
computes its 512x512 output chunk. No cross-core communication.

V2 changes vs the bf16 baseline:
- Q/K/V projections run as fp8e4 hi/lo-split DoubleRow matmuls: x is
  quantized (scale 8) to x_hi + x_lo, weights (scale 64) to W_hi +
  W_lo; the three first-order products accumulate in f32 PSUM with 2
  contraction k-tiles per instruction at 0.5 cycles/column (4x the
  bf16 MAC rate, accuracy better than bf16 since the lo terms cancel
  the quantization error). The 512*512 scale folds into the exp scale
  (2^-21, exact) and a host-side Wo/512 prescale.
- Even-head score matmuls contract K=64 directly (partitions 0:64),
  killing the even-head zero-masking; odd heads keep a zeroed-top
  qTo operand (tile_position row base 64 is rejected by walrus).
- Pool no longer does any head masking (was ~8us of gpsimd multiply);
  triangular mask tiles shrink to [mid|e0|selpair|ident] with the
  hb=0/hb=4 masks expressed as stride-0 broadcast APs over e0/mid.

Per-core attention (bf16 matmuls, fp32 PSUM accumulation), as in V1:
flattened software pipeline over (pair, k-block) stages with score
lookahead 2; sums via N=1 ones-matmuls; on-chip normalization via
tiny PE transposes -> DVE reciprocal -> K=2 broadcast matmul.
"""

from contextlib import ExitStack

import numpy as np
import ml_dtypes

import concourse.bacc as bacc
import concourse.tile as tile
import concourse.mybir as mybir
import concourse.bass as bass
from concourse import bass_utils

BF16 = mybir.dt.bfloat16
F32 = mybir.dt.float32
FP8 = mybir.dt.float8e4
DR = mybir.MatmulPerfMode.DoubleRow
E4NP = ml_dtypes.float8_e4m3

P = 128          # partitions / block size / window
S = 512          # chunk rows per core
SH = 640         # halo rows per core (128 + 512)
D = 512          # d_model
H = 8            # heads
DH = 64          # head dim
NKT = 4          # d_model tiles of 128
NPAIR = 4        # head pairs
NHB = 5          # halo k-blocks
N_CORES = 8
XSCALE = 8.0     # x fp8 pre-scale
WSCALE = 64.0    # weight fp8 pre-scale
# exp scale: 0.125 attention scale / (XSCALE*WSCALE)^2 = 2^-3 / 2^18
EXP_SCALE = 0.125 / float((XSCALE * WSCALE) ** 2)
# masks payload: 512 mid + 128 e0 + 128 selpair + 128 ident
MCOLS = 896

_nc_cache = None


def _build_kernel():
    nc = bacc.Bacc("TRN2", target_bir_lowering=False, debug=False,
                   enable_asserts=False)

    xhi_d = nc.dram_tensor("xhi", [P, NKT * SH], FP8, kind="ExternalInput")
    xlo_d = nc.dram_tensor("xlo", [P, NKT * SH], FP8, kind="ExternalInput")
    wqh_d = nc.dram_tensor("wqh", [P, NKT * D], FP8, kind="ExternalInput")
    wql_d = nc.dram_tensor("wql", [P, NKT * D], FP8, kind="ExternalInput")
    wkh_d = nc.dram_tensor("wkh", [P, NKT * D], FP8, kind="ExternalInput")
    wkl_d = nc.dram_tensor("wkl", [P, NKT * D], FP8, kind="ExternalInput")
    wvh_d = nc.dram_tensor("wvh", [P, NKT * D], FP8, kind="ExternalInput")
    wvl_d = nc.dram_tensor("wvl", [P, NKT * D], FP8, kind="ExternalInput")
    wo_d = nc.dram_tensor("wo", [P, NKT * D], BF16, kind="ExternalInput")
    masks_d = nc.dram_tensor("masks", [P, MCOLS], BF16, kind="ExternalInput")
    out_d = nc.dram_tensor("out", [S, D], BF16, kind="ExternalOutput")

    with tile.TileContext(nc) as tc, ExitStack() as ctx:
        _kernel_body(ctx, tc, xhi_d, xlo_d, wqh_d, wql_d, wkh_d, wkl_d,
                     wvh_d, wvl_d, wo_d, masks_d, out_d)
    nc.compile()
    return nc


def _kernel_body(ctx, tc, xhi_d, xlo_d, wqh_d, wql_d, wkh_d, wkl_d,
                 wvh_d, wvl_d, wo_d, masks_d, out_d):
    nc = tc.nc
    Exp = mybir.ActivationFunctionType.Exp

    persist = ctx.enter_context(tc.tile_pool(name="persist", bufs=1))
    expp = ctx.enter_context(tc.tile_pool(name="expp", bufs=7))
    rbcp = ctx.enter_context(tc.tile_pool(name="rbcp", bufs=2))
    pp512 = ctx.enter_context(tc.tile_pool(name="pp512", bufs=2, space="PSUM"))
    psc = ctx.enter_context(tc.tile_pool(name="psc", bufs=2, space="PSUM"))
    poT = ctx.enter_context(tc.tile_pool(name="poT", bufs=2, space="PSUM"))
    psums = ctx.enter_context(tc.tile_pool(name="psums", bufs=1, space="PSUM"))
    psnat = ctx.enter_context(tc.tile_pool(name="psnat", bufs=1, space="PSUM"))

    # memsets first: zero-fill matmuls then have no input dependencies
    zeros1 = persist.tile([1, 512], BF16, tag="zeros1")
    nc.gpsimd.memset(zeros1[:], 0.0)
    ones1 = persist.tile([P, 1], BF16, tag="ones1")
    nc.gpsimd.memset(ones1[:], 1.0)

    # ---------------- load inputs ----------------
    # One DMA per tensor on SP; transfers serialize on the shared DMA
    # device in issue order, so order by first use: Q path, K path, V.
    xhi = persist.tile([P, NKT * SH], FP8, tag="xhi")
    nc.sync.dma_start(xhi[:], xhi_d.ap()[:, :])
    wqh = persist.tile([P, NKT * D], FP8, tag="wqh")
    nc.sync.dma_start(wqh[:], wqh_d.ap()[:, :])
    xlo = persist.tile([P, NKT * SH], FP8, tag="xlo")
    nc.sync.dma_start(xlo[:], xlo_d.ap()[:, :])
    wql = persist.tile([P, NKT * D], FP8, tag="wql")
    nc.sync.dma_start(wql[:], wql_d.ap()[:, :])
    wkh = persist.tile([P, NKT * D], FP8, tag="wkh")
    nc.sync.dma_start(wkh[:], wkh_d.ap()[:, :])
    wkl = persist.tile([P, NKT * D], FP8, tag="wkl")
    nc.sync.dma_start(wkl[:], wkl_d.ap()[:, :])
    wvh = persist.tile([P, NKT * D], FP8, tag="wvh")
    nc.sync.dma_start(wvh[:], wvh_d.ap()[:, :])
    wvl = persist.tile([P, NKT * D], FP8, tag="wvl")
    nc.sync.dma_start(wvl[:], wvl_d.ap()[:, :])
    masks = persist.tile([P, MCOLS], BF16, tag="masks")
    nc.sync.dma_start(masks[:], masks_d.ap()[:, :])
    wo_sb = persist.tile([P, NKT * D], BF16, tag="wo")
    nc.sync.dma_start(wo_sb[:], wo_d.ap()[:, :])

    mask_mid = masks[:, 0:512]
    mask_e0 = masks[:, 512:640]      # [prev] or zeros on chunk-0 cores
    selpair = masks[0:2, 640:768]
    ident = masks[:, 768:896]
    wo_t = [wo_sb[:, k * D:(k + 1) * D] for k in range(NKT)]

    # 3D k-tile views for DoubleRow operands
    xhv = xhi[:].rearrange("p (k c) -> p k c", k=NKT)
    xlv = xlo[:].rearrange("p (k c) -> p k c", k=NKT)
    wqhv = wqh[:].rearrange("p (k c) -> p k c", k=NKT)
    wqlv = wql[:].rearrange("p (k c) -> p k c", k=NKT)
    wkhv = wkh[:].rearrange("p (k c) -> p k c", k=NKT)
    wklv = wkl[:].rearrange("p (k c) -> p k c", k=NKT)
    wvhv = wvh[:].rearrange("p (k c) -> p k c", k=NKT)
    wvlv = wvl[:].rearrange("p (k c) -> p k c", k=NKT)

    # early zero-fill of the first two pairs' AV accumulators: PE work with
    # no DMA dependencies, fills the input-load stall and warms the ramp
    def alloc_oT(extra=0):
        op = poT.tile([P, 512], F32, tag="oT", name="oTps")
        nc.tensor.matmul(op[:], zeros1[0:1, 0:P], zeros1[0:1, 0:512],
                         start=True, stop=False, skip_group_check=True)
        for _ in range(extra):  # ramp warmers: accumulate more zeros
            nc.tensor.matmul(op[:], zeros1[0:1, 0:P], zeros1[0:1, 0:512],
                             start=False, stop=False, skip_group_check=True)
        return op

    pair_oT = {0: alloc_oT(extra=1), 1: alloc_oT(extra=0)}

    # ---------------- projections (fp8 hi/lo DoubleRow) ----------------
    # Each output accumulates 3 first-order terms x 2 k-tile pairs in PSUM.
    # Term order matches DMA arrival: (hi,hi) -> (lo-x,hi-w) -> (hi-x,lo-w).
    def proj_terms(ps_ap, w3, x3, xc0, xc1, mc0, mc1, x_stationary=False):
        terms = [(w3[0], x3[0]), (w3[0], x3[1]), (w3[1], x3[0])]
        n = 0
        for wv_, xv_ in terms:
            for kp in range(2):
                n += 1
                if x_stationary:
                    lhsT = xv_[:, 2 * kp:2 * kp + 2, xc0:xc1]
                    rhs = wv_[:, 2 * kp:2 * kp + 2, mc0:mc1]
                else:
                    lhsT = wv_[:, 2 * kp:2 * kp + 2, mc0:mc1]
                    rhs = xv_[:, 2 * kp:2 * kp + 2, xc0:xc1]
                nc.tensor.matmul(ps_ap, lhsT, rhs, start=(n == 1),
                                 stop=(n == 6), perf_mode=DR)

    # qT[m]: [128, 512] chunk cols; qTe = even-head rows only [64, 512],
    # qTo = odd-head rows with zeroed top half (for K=128 odd score matmul)
    qTe_sb, qTo_sb = [], []
    for m in range(NKT):
        to = persist.tile([P, 512], BF16, tag=f"qTo{m}", name=f"qTo{m}")
        nc.gpsimd.memset(to[0:DH, :], 0.0)
        ps = pp512.tile([P, 512], F32, tag="ps512", name="ps512")
        proj_terms(ps[:], (wqhv, wqlv), (xhv, xlv), P, SH,
                   m * P, (m + 1) * P)
        te = persist.tile([DH, 512], BF16, tag=f"qTe{m}", name=f"qTe{m}")
        nc.scalar.copy(te[:], ps[0:DH, :])
        nc.vector.tensor_copy(to[DH:P, :], ps[DH:P, :])
        qTe_sb.append(te)
        qTo_sb.append(to)

    # warmers between qT and kT: fill the wk-load stall, keep the ramp hot
    nc.tensor.matmul(pair_oT[0][:], zeros1[0:1, 0:P], zeros1[0:1, 0:512],
                     start=False, stop=False, skip_group_check=True)
    nc.tensor.matmul(pair_oT[1][:], zeros1[0:1, 0:P], zeros1[0:1, 0:512],
                     start=False, stop=False, skip_group_check=True)

    # kT[m]: [128, 640] halo cols; 512-wide + 128-wide PSUM pieces
    kT_sb = []
    for m in range(NKT):
        t = persist.tile([P, SH], BF16, tag=f"kT{m}", name=f"kT{m}")
        ps = pp512.tile([P, 512], F32, tag="ps512", name="ps512")
        proj_terms(ps[:], (wkhv, wklv), (xhv, xlv), 0, 512,
                   m * P, (m + 1) * P)
        nc.scalar.copy(t[:, 0:512], ps[:])
        ps2 = pp512.tile([P, 512], F32, tag="ps512", name="ps512")
        proj_terms(ps2[:, 0:P], (wkhv, wklv), (xhv, xlv), 512, SH,
                   m * P, (m + 1) * P)
        nc.vector.tensor_copy(t[:, 512:SH], ps2[:, 0:P])
        kT_sb.append(t)

    # ---------------- attention setup ----------------
    oT_sb = []
    # persistent natural-layout sums for all pairs (one bank, single-shot
    # column writes; interleaved open accumulation groups corrupt PSUM)
    s_all = psnat.tile([P, 64], F32, tag="snat", name="s_all")

    def hbdims(hb):
        pc0 = max(0, (hb - 1) * P)
        pc1 = min(512, (hb + 1) * P)
        return pc0, pc1, pc1 - pc0

    et_tiles = {}

    def emit_sc(t, hb):
        """Scores + exp + mask for stage (t, hb), emitted 2 stages ahead:
        the ACT/Pool chain drains while earlier stages' PE work runs, and
        the psc slot frees early enough that lookahead never blocks."""
        pc0, pc1, pw = hbdims(hb)
        sc = psc.tile([P, 512], F32, tag="sc", name="sc")
        # even head: direct K=64 contraction over partitions 0:64
        nc.tensor.matmul(sc[:, 0:pw], kT_sb[t][0:DH, hb * P:(hb + 1) * P],
                         qTe_sb[t][:, pc0:pc1], start=True, stop=True)
        # odd head: K=128 against zero-topped qTo
        nc.tensor.matmul(sc[:, pw:2 * pw], kT_sb[t][:, hb * P:(hb + 1) * P],
                         qTo_sb[t][:, pc0:pc1], start=True, stop=True)
        # exp( scores * 0.125 / (512*512) ) -- fp8 pre-scales fold out here
        et = expp.tile([P, 512], BF16, tag="expT", name="expT")
        nc.scalar.activation(et[:, 0:2 * pw], sc[:, 0:2 * pw], Exp,
                             scale=EXP_SCALE)
        # multiplicative triangular mask (hb 0/4 via stride-0 broadcast)
        if hb == 0:
            mask = mask_e0.rearrange("p (a c) -> p a c", a=1) \
                .broadcast_to((P, 2, P))
            etv = et[:, 0:2 * pw].rearrange("p (a c) -> p a c", a=2)
            nc.gpsimd.tensor_mul(etv, etv, mask)
        elif hb == NHB - 1:
            mask = mask_mid[:, 0:P].rearrange("p (a c) -> p a c", a=1) \
                .broadcast_to((P, 2, P))
            etv = et[:, 0:2 * pw].rearrange("p (a c) -> p a c", a=2)
            nc.gpsimd.tensor_mul(etv, etv, mask)
        else:
            nc.gpsimd.tensor_mul(et[:, 0:2 * pw], et[:, 0:2 * pw],
                                 mask_mid[:, 0:2 * pw])
        et_tiles[(t, hb)] = et

    # first two score tiles precede the V projection on the in-order PE
    # queue: the exp/mask/sums front-end warms up while V waits for wvl
    emit_sc(0, 0)
    emit_sc(0, 1)
    emit_sc(0, 2)
    emit_sc(0, 3)
    emit_sc(0, 4)

    # v[st]: [128, 512] natural rows; x stationary, wv moving
    v_sb = []
    for st in range(NHB):
        ps = pp512.tile([P, 512], F32, tag="ps512", name="ps512")
        proj_terms(ps[:], (wvhv, wvlv), (xhv, xlv), st * P, (st + 1) * P,
                   0, D, x_stationary=True)
        t = persist.tile([P, 512], BF16, tag=f"v{st}", name=f"v{st}")
        nc.vector.tensor_copy(t[:], ps[:])
        v_sb.append(t)

    # ---------------- attention stages ----------------
    # Flattened software pipeline over (pair, k-block) stages with score
    # lookahead 2: scores for stage i+2 are emitted before stage i's AV so
    # the in-order PE queue never head-of-line blocks on exp+mask latency.
    stages = [(t, hb) for t in range(NPAIR) for hb in range(NHB)]
    oT_ps = None
    # out-projection accumulators, filled in as pairs complete (see below)
    fps = {}
    pending_norm = {}
    pair_ps = {}

    def fmm(f, mt, t):
        nc.tensor.matmul(f[:], oT_sb[t][:, mt * P:(mt + 1) * P],
                         wo_t[t][:], start=(t == 0), stop=(t == NPAIR - 1))

    def finish_norm(t):
        # broadcast 1/sums across the pair's head rows, then normalize;
        # the DVE multiply reads both operands straight from PSUM
        rinv = pending_norm.pop(t)
        bc = psums.tile([P, 512], F32, tag="sums", name="bc")
        nc.tensor.matmul(bc[:], selpair[:], rinv[:], start=True, stop=True)
        bcs = rbcp.tile([P, 512], BF16, tag="bcs", name="bcs")
        ot = persist.tile([P, 512], BF16, tag=f"oT{t}", name=f"oTsb{t}")
        ps = pair_ps.pop(t)
        if t == NPAIR - 1:
            # last pair: per-m-tile chunks, m3 first, so each fmm/fout in
            # the tail starts as soon as its 128 columns are normalized
            for mt in (3, 2, 1, 0):
                c0, c1 = mt * P, (mt + 1) * P
                nc.scalar.copy(bcs[:, c0:c1], bc[:, c0:c1])
                nc.vector.tensor_mul(ot[:, c0:c1], ps[:, c0:c1],
                                     bcs[:, c0:c1])
        else:
            nc.scalar.copy(bcs[:], bc[:])
            nc.vector.tensor_mul(ot[:], ps[:], bcs[:])
        oT_sb.append(ot)

    for i, (t, hb) in enumerate(stages):
        h0, h1 = 2 * t, 2 * t + 1
        pc0, pc1, pw = hbdims(hb)
        last = (hb == NHB - 1)
        if hb == 0:
            if t >= 1:
                finish_norm(t - 1)
            oT_ps = pair_oT[t] if t in pair_oT else alloc_oT()
        et = et_tiles.pop((t, hb))
        # sums: single-shot N=1 ones-matmuls per (covered qblock, head);
        # col layout within the pair's 16-col region: (qb*2+h)*2 + idx
        sb0 = t * 16
        for h, hoff in ((0, 0), (1, pw)):
            if hb >= 1:  # qblock hb-1, curr-block contribution
                c = sb0 + ((hb - 1) * 2 + h) * 2 + 1
                nc.tensor.matmul(s_all[:, c:c + 1],
                                 et[:, hoff:hoff + P], ones1[:, 0:1],
                                 start=True, stop=True, skip_group_check=True)
            if hb <= 3:  # qblock hb, prev-block contribution
                c0 = hoff + (P if hb >= 1 else 0)
                c = sb0 + (hb * 2 + h) * 2
                nc.tensor.matmul(s_all[:, c:c + 1],
                                 et[:, c0:c0 + P], ones1[:, 0:1],
                                 start=True, stop=True, skip_group_check=True)
        if last:
            # normalize front-end before the final AV matmuls so the
            # reciprocal chain overlaps them; pair sums are read straight
            # out of PSUM (single-shot columns, safe to read). The bc/bcs
            # broadcast + the oT multiply are deferred to the next pair's
            # first stage (finish_norm) so the bc matmul never head-of-line
            # blocks this pair's remaining PE work on the DVE reciprocal.
            s16 = rbcp.tile([P, 16], F32, tag="s16", name="s16")
            nc.vector.tensor_copy(s16[:], s_all[:, sb0:sb0 + 16])
            ssb = rbcp.tile([P, 8], BF16, tag="ssb", name="ssb")
            s2 = s16[:].rearrange("p (a b) -> p a b", b=2)
            nc.vector.tensor_add(ssb[:], s2[:, :, 0], s2[:, :, 1])
            rT = psums.tile([2, 512], F32, tag="sums", name="rT")
            for b in range(4):
                # transpose-by-matmul: out[h,qp] = sum_k ssb[k,2b+h]*I[k,qp]
                nc.tensor.matmul(rT[0:2, b * P:(b + 1) * P],
                                 ssb[:, b * 2:b * 2 + 2], ident[:],
                                 start=True, stop=True)
            rinv = rbcp.tile([2, 512], BF16, tag="rinv", name="rinv")
            with nc.allow_low_precision(reason="bf16 1/sums -> bf16 mm"):
                nc.vector.reciprocal(rinv[:], rT[:])
            pending_norm[t] = rinv
        # score lookahead before this stage's (potentially blocking) AV
        if i + 5 < len(stages):
            emit_sc(*stages[i + 5])
        # out-projection fmms for completed pairs interleave here (hb==2:
        # far enough past the pair boundary that ot[t-1] is surely ready,
        # so these never head-of-line block the stage pipeline)
        if hb == 2 and t >= 1:
            if t == 1:
                fps[0] = pp512.tile([P, 512], F32, tag="ps512", name="fps0")
                fps[1] = pp512.tile([P, 512], F32, tag="ps512", name="fps1")
            for mt in (0, 1):
                fmm(fps[mt], mt, t - 1)
            if t == 3:
                # pair 2's oT slot freed at its normalize; reuse for m2
                fps[2] = poT.tile([P, 512], F32, tag="oT", name="fps2")
                for tt in (0, 1, 2):
                    fmm(fps[2], 2, tt)
        # attn @ v: transposed head outputs, col-group packed pair,
        # overlapping-range PSUM accumulation (zero-fill MM opened group)
        nc.tensor.matmul(oT_ps[0:DH, pc0:pc1],
                         v_sb[hb][:, h0 * DH:(h0 + 1) * DH], et[:, 0:pw],
                         start=False, stop=last, tile_position=(0, 0),
                         skip_group_check=True)
        nc.tensor.matmul(oT_ps[DH:P, pc0:pc1],
                         v_sb[hb][:, h1 * DH:(h1 + 1) * DH], et[:, pw:2 * pw],
                         start=False, stop=last, tile_position=(0, 64),
                         skip_group_check=True)
        if last:
            pair_ps[t] = oT_ps

    # ---------------- output projection tail ----------------
    # m0-m2 accumulated pairs 0-2 inside the stage loop; here only m3's
    # pair 0-2 matmuls (they fill the pair-3 normalize bubble) and the
    # four t=3 matmuls + copies + DMAs remain. m3 first so its DMA -- the
    # longest pole -- dispatches earliest.
    def fout(f, mt):
        osb = rbcp.tile([P, 512], BF16, tag="osb", name="osb", bufs=4)
        if mt % 2 == 0:
            nc.vector.tensor_copy(osb[:], f[:])
        else:
            nc.scalar.copy(osb[:], f[:])
        nc.sync.dma_start(out_d.ap()[mt * P:(mt + 1) * P, :], osb[:])

    # m3 accumulator reuses a psc slot (freed 2 stages ago by the exp
    # lookahead); its pair 0-2 matmuls fill the pair-3 normalize bubble
    fps[3] = psc.tile([P, 512], F32, tag="sc", name="fps3")
    for tt in (0, 1, 2):
        fmm(fps[3], 3, tt)
    finish_norm(3)
    for mt in (3, 2, 1, 0):
        fmm(fps[mt], mt, 3)
    for mt in (3, 2, 1, 0):
        fout(fps[mt], mt)


def _get_nc():
    global _nc_cache
    if _nc_cache is None:
        _nc_cache = _build_kernel()
    return _nc_cache


def _make_masks(zero_e0):
    j = np.arange(P)[:, None]
    c = np.arange(P)[None, :]
    curr = (j <= c).astype(ml_dtypes.bfloat16)   # k-block == q-block
    prev = (j > c).astype(ml_dtypes.bfloat16)    # k-block == q-block - 1
    mask_mid = np.concatenate([curr, prev, curr, prev], axis=1)
    e0 = np.zeros_like(prev) if zero_e0 else prev
    selpair = np.zeros((P, P), dtype=ml_dtypes.bfloat16)
    selpair[0, 0:DH] = 1.0
    selpair[1, DH:P] = 1.0
    ident = np.eye(P, dtype=ml_dtypes.bfloat16)
    return np.concatenate([mask_mid, e0, selpair, ident], axis=1)


def _pack_ktile_major(a, cols):
    """[512 rows, cols] -> [128, 4*cols] with k-tile k at column block k."""
    return np.ascontiguousarray(
        a.reshape(NKT, P, cols).transpose(1, 0, 2).reshape(P, NKT * cols))


def _split8(a, scale):
    hi = np.clip(a * scale, -240.0, 240.0).astype(E4NP)
    lo = (a * scale - hi.astype(np.float32)).astype(E4NP)
    return hi, lo


def _prep_inputs(x, Wq, Wk, Wv, Wo):
    x = np.asarray(x, dtype=np.float32)
    masks_all = _make_masks(zero_e0=False)
    masks_z = _make_masks(zero_e0=True)
    w8 = {}
    for name, W in (("wq", Wq), ("wk", Wk), ("wv", Wv)):
        hi, lo = _split8(np.asarray(W, np.float32), WSCALE)
        w8[name + "h"] = _pack_ktile_major(hi, D)
        w8[name + "l"] = _pack_ktile_major(lo, D)
    wo = _pack_ktile_major(
        (np.asarray(Wo, np.float32) / (XSCALE * WSCALE)).astype(
            ml_dtypes.bfloat16), D)
    in_maps = []
    for core in range(N_CORES):
        b, chunk = divmod(core, 4)
        c0 = chunk * S
        xh = np.zeros((SH, D), np.float32)
        lo_r = c0 - P
        src_lo = max(0, lo_r)
        xh[src_lo - lo_r:, :] = x[b, src_lo:c0 + S, :]
        xT = np.ascontiguousarray(xh.T)               # [D, SH]
        xT_hi, xT_lo = _split8(xT, XSCALE)
        in_maps.append({
            "xhi": _pack_ktile_major(xT_hi, SH),
            "xlo": _pack_ktile_major(xT_lo, SH),
            "wqh": w8["wqh"], "wql": w8["wql"],
            "wkh": w8["wkh"], "wkl": w8["wkl"],
            "wvh": w8["wvh"], "wvl": w8["wvl"],
            "wo": wo,
            "masks": masks_z if chunk == 0 else masks_all,
        })
    return in_maps


def kernel(x, Wq, Wk, Wv, Wo, _profile=None):
    nc = _get_nc()
    in_maps = _prep_inputs(x, Wq, Wk, Wv, Wo)
    res = bass_utils.run_bass_kernel_spmd(nc, in_maps,
                                          core_ids=list(range(N_CORES)))
    x = np.asarray(x)
    B, S_full, _ = x.shape
    out = np.empty((B, S_full, D), np.float32)
    for core in range(N_CORES):
        b, chunk = divmod(core, 4)
        out[b, chunk * S:(chunk + 1) * S, :] = (
            res.results[core]["out"].astype(np.float32))
    if _profile is not None:
        _profile.append(res)
    return out
